# revision 8
# baseline (speedup 1.0000x reference)
"""Trainium2 Bass kernel for an attention-decoder LSTM (nn_Decoder).

Data-parallel over 8 NeuronCores: batch 4096 -> 512 per core. All weights
replicated. The T-1=127 step recurrence runs fully on-chip: enc_proj is
precomputed once into SBUF (bf16, [ENC, T, B] layout) and every step does
  hp   = 0.5*W1_h.T @ H + 0.5*W1_c.T @ C          (PE, H=2h, C=2c)
  X    = tanh(enc_proj + hp)                       (DVE add + ACT tanh)
  e    = w2.T @ X      -> PSUM rows [t, b]         (PE, one-hot shift stationary)
  S    = exp(e)                                    (ACT)
  den  = ones.T @ S ; num = ones.T @ (S*pfc)       (PE)
  r    = num * recip_fast(den)                     (DVE)
  gates= s_g*(0.5*W_hh.T @ H) [early] + s_g*(W_ih (x) yt + bias)  (PE)
  two fused tanh over [D, 2B] PSUM, LSTM update in tanh-only form
Final output row: 0.5*Wfh.T @ H + (ones.T @ (S*pfin))/den + fc_final_b.

Schedule notes (v1): ACT is the bottleneck engine (~60us/step busy floor).
X tiles are 4-buffered so DVE ADD runs ahead of ACT tanh; chunk sizes are
tapered [4, 8x15, 4] to restart/finish the pipeline quickly; the W_hh gate
matmuls are issued at the end of the previous step (they only need H) so the
tail only waits on the tiny W_ih rank-1 matmuls; the gate biases are folded
into the matmul via an augmented [yt; ones] moving operand.
"""

import numpy as np
import ml_dtypes

import concourse.bass as bass
import concourse.bacc as bacc
import concourse.tile as tile
from concourse import mybir
from concourse.bass_utils import run_bass_kernel_spmd

NCORES = 8
B_FULL, T, E, D = 4096, 128, 128, 128
B = B_FULL // NCORES        # 512 batch per core
TSTEPS = T - 1              # 127

FP = mybir.dt.float32
BF = mybir.dt.bfloat16
AF = mybir.ActivationFunctionType
OP = mybir.AluOpType
BF_NP = ml_dtypes.bfloat16

# tapered t-chunks for the big add/tanh passes
CHUNKS = []
_t0 = 0
for _sz in [4] + [8] * 15 + [4]:
    CHUNKS.append((_t0, _sz))
    _t0 += _sz
assert _t0 == T


def _build(fc_wy: float, fc_final_b: float, n_steps: int):
    nc = bacc.Bacc("TRN2", target_bir_lowering=False, debug=False,
                   num_devices=NCORES)

    x_ext = nc.declare_dram_parameter("x", [B, T, E], FP, isOutput=False)
    yh_ext = nc.declare_dram_parameter("yh", [max(TSTEPS, 1), B], BF,
                                       isOutput=False)
    # [0.5*W1_h.T | 0.5*W1_c.T]  -> [D, 2E]
    w1hc_ext = nc.declare_dram_parameter("w1hc", [D, 2 * E], BF, isOutput=False)
    wke_ext = nc.declare_dram_parameter("wke", [E, E], BF, isOutput=False)  # W1_e.T
    # shifted one-hot stationaries: zeros except column 127 = vec
    w2g_ext = nc.declare_dram_parameter("w2g", [E, 2 * T], BF, isOutput=False)
    gfc_ext = nc.declare_dram_parameter("gfc", [E, 2 * T], BF, isOutput=False)
    gfin_ext = nc.declare_dram_parameter("gfin", [E, 2 * T], BF, isOutput=False)
    # per-gate pre-scaled: col block g = s_g*0.5*W_hh.T  (s_i,f,o=0.5, s_g=1)
    whh_ext = nc.declare_dram_parameter("whh", [D, 4 * D], BF, isOutput=False)
    # row0 = s_g*W_ih col, row1 = s_g*(b_ih+b_hh+W_ih*fc_b)
    wihb_ext = nc.declare_dram_parameter("wihb", [2, 4 * D], BF, isOutput=False)
    b1_ext = nc.declare_dram_parameter("b1", [E, 1], FP, isOutput=False)
    wfh_ext = nc.declare_dram_parameter("wfh", [D, 1], BF, isOutput=False)  # 0.5*Wfh
    id_ext = nc.declare_dram_parameter("ident", [128, 128], BF, isOutput=False)
    out_ext = nc.declare_dram_parameter("out", [1, B], FP, isOutput=True)

    with tile.TileContext(nc) as tc:
        import contextlib
        _stack = contextlib.ExitStack()
        const = _stack.enter_context(tc.tile_pool(name="const", bufs=1))
        state = _stack.enter_context(tc.tile_pool(name="state", bufs=2))
        xpool = _stack.enter_context(tc.tile_pool(name="xpool", bufs=4))
        tw = _stack.enter_context(tc.tile_pool(name="tw", bufs=1))
        intp = _stack.enter_context(tc.tile_pool(name="intp", bufs=2))
        dma4 = _stack.enter_context(tc.tile_pool(name="dma4", bufs=4))
        # PSUM: gA(2) + gB(2) + e(1) + hp(1) + den(1) + num(1) = 8 banks
        psg = _stack.enter_context(tc.tile_pool(name="psg", bufs=1, space="PSUM"))
        pse = _stack.enter_context(tc.tile_pool(name="pse", bufs=1, space="PSUM"))
        psh = _stack.enter_context(tc.tile_pool(name="psh", bufs=1, space="PSUM"))
        psm = _stack.enter_context(tc.tile_pool(name="psm", bufs=1, space="PSUM"))

        # ---- constants -------------------------------------------------
        w1hc_sb = const.tile([D, 2 * E], BF, tag="w1hc")
        nc.sync.dma_start(out=w1hc_sb[:], in_=w1hc_ext[:])
        wke_sb = const.tile([E, E], BF, tag="wke")
        nc.sync.dma_start(out=wke_sb[:], in_=wke_ext[:])
        w2g_sb = const.tile([E, 2 * T], BF, tag="w2g")
        nc.sync.dma_start(out=w2g_sb[:], in_=w2g_ext[:])
        gfc_sb = const.tile([E, 2 * T], BF, tag="gfc")
        nc.sync.dma_start(out=gfc_sb[:], in_=gfc_ext[:])
        gfin_sb = const.tile([E, 2 * T], BF, tag="gfin")
        nc.sync.dma_start(out=gfin_sb[:], in_=gfin_ext[:])
        whh_sb = const.tile([D, 4 * D], BF, tag="whh")
        nc.sync.dma_start(out=whh_sb[:], in_=whh_ext[:])
        wihb_sb = const.tile([2, 4 * D], BF, tag="wihb")
        nc.sync.dma_start(out=wihb_sb[:], in_=wihb_ext[:])
        b1_sb = const.tile([E, 1], FP, tag="b1")
        nc.sync.dma_start(out=b1_sb[:], in_=b1_ext[:])
        wfh_sb = const.tile([D, 1], BF, tag="wfh")
        nc.sync.dma_start(out=wfh_sb[:], in_=wfh_ext[:])
        id_sb = const.tile([128, 128], BF, tag="ident")
        nc.sync.dma_start(out=id_sb[:], in_=id_ext[:])
        ones_sb = const.tile([T, 1], BF, tag="ones")
        nc.vector.memset(ones_sb[:], 1.0)
        # moving operand for the W_ih/bias gate matmul: row0 = yt, row1 = 1
        ytones = const.tile([2, B], BF, tag="ytones")
        nc.vector.memset(ytones[:], 1.0)

        encp = const.tile([E, T, B], BF, tag="encp")
        pfc_sb = const.tile([T, B], BF, tag="pfc")
        pfin_sb = const.tile([T, B], BF, tag="pfin")
        C = const.tile([D, B], FP, tag="C")   # 2*c
        nc.vector.memset(C[:], 0.0)

        # ---- precompute: enc_proj, pfc, pfin ---------------------------
        pfc_ps = psm.tile([T, B], FP, tag="den")
        pfin_ps = psm.tile([T, B], FP, tag="num")
        for t in range(T):
            inT_ps = psg.tile([E, B], BF, tag="gA")
            for blk in range(B // 128):
                xin = dma4.tile([128, E], FP, tag="xin")
                nc.sync.dma_start(
                    out=xin[:],
                    in_=x_ext[blk * 128:(blk + 1) * 128, t, :],
                )
                xbf = dma4.tile([128, E], BF, tag="xbf")
                nc.vector.tensor_copy(xbf[:], xin[:])
                nc.tensor.transpose(
                    inT_ps[:, blk * 128:(blk + 1) * 128], xbf[:], id_sb[:]
                )
            inT = intp.tile([E, B], BF, tag="inT")
            nc.vector.tensor_copy(inT[:], inT_ps[:])
            ep_ps = psg.tile([E, B], FP, tag="gB")
            nc.tensor.matmul(ep_ps[:], wke_sb[:], inT[:],
                             start=True, stop=True)
            nc.tensor.matmul(pfc_ps[:], gfc_sb[:, T - 1 - t:2 * T - 1 - t],
                             inT[:], start=(t == 0), stop=(t == T - 1))
            nc.tensor.matmul(pfin_ps[:], gfin_sb[:, T - 1 - t:2 * T - 1 - t],
                             inT[:], start=(t == 0), stop=(t == T - 1))
            # enc_proj + attn_b1, cast to bf16, store [E, t, B]
            nc.scalar.activation(encp[:, t, :], ep_ps[:],
                                 AF.Identity, bias=b1_sb[:], scale=1.0)
        nc.vector.tensor_copy(pfc_sb[:], pfc_ps[:])
        nc.vector.tensor_copy(pfin_sb[:], pfin_ps[:])

        # initial state (zeros)
        Hbf = state.tile([D, B], BF, tag="Hbf")
        Cbf = state.tile([D, B], BF, tag="Cbf")
        nc.vector.memset(Hbf[:], 0.0)
        nc.vector.memset(Cbf[:], 0.0)
        hp_sb = state.tile([E, B], BF, tag="hp")
        nc.vector.memset(hp_sb[:], 0.0)

        # gate PSUM tiles for step 0 + early W_hh matmuls (H=0 -> zeros, but
        # keeps the per-step structure uniform)
        gA = psg.tile([D, 2 * B], FP, tag="gA")
        gB = psg.tile([D, 2 * B], FP, tag="gB")
        for g in range(4):
            gt = gA if g < 2 else gB
            off = (g % 2) * B
            nc.tensor.matmul(gt[:, off:off + B], whh_sb[:, g * D:(g + 1) * D],
                             Hbf[:], start=True, stop=False)

        rcp = None
        S_sb = None

        # ---- the recurrence -------------------------------------------
        for s in range(n_steps):
            # X phase: X = tanh(encp + hp), e rows via one-hot matmuls
            e_ps = pse.tile([T, B], FP, tag="e")
            for (t0, tcsz) in CHUNKS:
                X = xpool.tile([E, 8, B], BF, tag="X")
                hp_b = hp_sb[:].unsqueeze(1).broadcast_to([E, tcsz, B])
                nc.vector.tensor_tensor(
                    X[:, :tcsz, :], encp[:, t0:t0 + tcsz, :], hp_b, op=OP.add
                )
                nc.scalar.activation(X[:, :tcsz, :], X[:, :tcsz, :], AF.Tanh)
                for j in range(tcsz):
                    t = t0 + j
                    nc.tensor.matmul(e_ps[:], w2g_sb[:, T - 1 - t:2 * T - 1 - t],
                                     X[:, j, :], start=(t == 0), stop=(t == T - 1))

            # softmax-weighted average r = sum(S*pfc)/sum(S)
            S_sb = tw.tile([T, B], BF, tag="S")
            nc.scalar.activation(S_sb[:], e_ps[:], AF.Exp)
            den_ps = psm.tile([1, B], FP, tag="den")
            nc.tensor.matmul(den_ps[:], ones_sb[:], S_sb[:],
                             start=True, stop=True)
            SP = tw.tile([T, B], BF, tag="SP")
            nc.vector.tensor_tensor(SP[:], S_sb[:], pfc_sb[:], op=OP.mult)
            num_ps = psm.tile([1, B], FP, tag="num")
            nc.tensor.matmul(num_ps[:], ones_sb[:], SP[:],
                             start=True, stop=True)
            rcp = tw.tile([1, B], FP, tag="rcp")
            nc.vector.reciprocal_approx_fast(rcp[:], den_ps[:])
            r = tw.tile([1, B], FP, tag="r")
            nc.vector.tensor_tensor(r[:], num_ps[:], rcp[:], op=OP.mult)
            yrow = dma4.tile([1, B], BF, tag="yrow")
            nc.sync.dma_start(out=yrow[:], in_=yh_ext[s:s + 1, :])
            # yt row (sans fc_b, folded into gate bias row)
            nc.vector.scalar_tensor_tensor(ytones[0:1, :], yrow[:],
                                           fc_wy, r[:], op0=OP.mult, op1=OP.add)

            # finish gates: += s_g*(W_ih (x) yt + bias)
            for g in range(4):
                gt = gA if g < 2 else gB
                off = (g % 2) * B
                nc.tensor.matmul(gt[:, off:off + B],
                                 wihb_sb[:, g * D:(g + 1) * D], ytones[:],
                                 start=False, stop=True)
            tgA = tw.tile([D, 2 * B], FP, tag="tgA")   # [ti | tf]
            nc.scalar.activation(tgA[:], gA[:], AF.Tanh)
            tgB = tw.tile([D, 2 * B], FP, tag="tgB")   # [tg | to]
            nc.scalar.activation(tgB[:], gB[:], AF.Tanh)

            # C_new(=2c) = 0.5*(tf+1)*C + (ti+1)*tg ; H_new(=2h) = (to+1)*tanh(c)
            tmp1 = tw.tile([D, B], FP, tag="tmp1")
            nc.vector.scalar_tensor_tensor(tmp1[:], tgA[:, B:2 * B], 1.0, C[:],
                                           op0=OP.add, op1=OP.mult)
            tmp2 = tw.tile([D, B], FP, tag="tmp2")
            nc.vector.scalar_tensor_tensor(tmp2[:], tgA[:, 0:B], 1.0,
                                           tgB[:, 0:B], op0=OP.add, op1=OP.mult)
            nc.vector.scalar_tensor_tensor(C[:], tmp1[:], 0.5, tmp2[:],
                                           op0=OP.mult, op1=OP.add)
            tct = tw.tile([D, B], FP, tag="tct")
            nc.scalar.activation(tct[:], C[:], AF.Tanh, scale=0.5)
            Hbf = state.tile([D, B], BF, tag="Hbf")
            nc.vector.scalar_tensor_tensor(Hbf[:], tgB[:, B:2 * B], 1.0, tct[:],
                                           op0=OP.add, op1=OP.mult)
            Cbf = state.tile([D, B], BF, tag="Cbf")
            nc.vector.tensor_copy(Cbf[:], C[:])

            if s + 1 < n_steps:
                # next-step hp + early W_hh gate matmuls
                hp_ps = psh.tile([E, B], FP, tag="hp")
                nc.tensor.matmul(hp_ps[:], w1hc_sb[:, E:2 * E], Cbf[:],
                                 start=True, stop=False)
                nc.tensor.matmul(hp_ps[:], w1hc_sb[:, 0:E], Hbf[:],
                                 start=False, stop=True)
                hp_sb = state.tile([E, B], BF, tag="hp")
                nc.vector.tensor_copy(hp_sb[:], hp_ps[:])
                gA = psg.tile([D, 2 * B], FP, tag="gA")
                gB = psg.tile([D, 2 * B], FP, tag="gB")
                for g in range(4):
                    gt = gA if g < 2 else gB
                    off = (g % 2) * B
                    nc.tensor.matmul(gt[:, off:off + B],
                                     whh_sb[:, g * D:(g + 1) * D],
                                     Hbf[:], start=True, stop=False)

        # ---- final output row ----------------------------------------
        o_ps = psm.tile([1, B], FP, tag="den")
        nc.tensor.matmul(o_ps[:], wfh_sb[:], Hbf[:], start=True, stop=True)
        if n_steps > 0:
            SPf = tw.tile([T, B], BF, tag="SP")
            nc.vector.tensor_tensor(SPf[:], S_sb[:], pfin_sb[:], op=OP.mult)
            nf_ps = psm.tile([1, B], FP, tag="num")
            nc.tensor.matmul(nf_ps[:], ones_sb[:], SPf[:], start=True, stop=True)
            rfin = tw.tile([1, B], FP, tag="r")
            nc.vector.tensor_tensor(rfin[:], nf_ps[:], rcp[:], op=OP.mult)
            o_sb = tw.tile([1, B], FP, tag="osb")
            nc.vector.scalar_tensor_tensor(o_sb[:], o_ps[:], fc_final_b, rfin[:],
                                           op0=OP.add, op1=OP.add)
        else:
            o_sb = tw.tile([1, B], FP, tag="osb")
            nc.vector.tensor_scalar_add(o_sb[:], o_ps[:], fc_final_b)
        nc.sync.dma_start(out=out_ext[:], in_=o_sb[:])
        _stack.close()

    nc.finalize()
    return nc


def _prep_host(inputs, n_steps):
    f32 = np.float32
    attn_W1 = np.asarray(inputs["attn_W1"], f32)
    attn_W2 = np.asarray(inputs["attn_W2"], f32)
    W_ih = np.asarray(inputs["W_ih"], f32)
    W_hh = np.asarray(inputs["W_hh"], f32)
    b_ih = np.asarray(inputs["b_ih"], f32)
    b_hh = np.asarray(inputs["b_hh"], f32)
    fc_W = np.asarray(inputs["fc_W"], f32)
    fc_b = np.asarray(inputs["fc_b"], f32)
    fcf_W = np.asarray(inputs["fc_final_W"], f32)
    fcf_b = np.asarray(inputs["fc_final_b"], f32)

    W1_h = attn_W1[:, :D]
    W1_c = attn_W1[:, D:2 * D]
    W1_e = attn_W1[:, 2 * D:]

    w1hc = np.concatenate([0.5 * W1_h.T, 0.5 * W1_c.T], axis=1)      # [D, 2E]
    wke = np.ascontiguousarray(W1_e.T)                                # [E, E]
    def onehot_shift(vec):
        g = np.zeros((E, 2 * T), f32)
        g[:, T - 1] = vec
        return g.astype(BF_NP)
    w2g = onehot_shift(attn_W2[0])
    gfc = onehot_shift(fc_W[0, :E])
    gfin = onehot_shift(fcf_W[0, D:])
    fc_wy = float(fc_W[0, E])
    wfh = 0.5 * fcf_W[0, :D][:, None]                                 # [D, 1]

    scales = np.array([0.5, 0.5, 1.0, 0.5], f32)
    gate_scale = np.repeat(scales, D)                                 # [4D]
    whh = (0.5 * W_hh.T) * gate_scale[None, :]                        # [D, 4D]
    bs = b_ih + b_hh + W_ih[:, 0] * float(fc_b[0])                    # [4D]
    wihb = np.stack([W_ih[:, 0] * gate_scale, bs * gate_scale], axis=0)  # [2, 4D]
    b1 = np.asarray(inputs["attn_b1"], f32)[:, None]

    weights = {
        "w1hc": w1hc.astype(BF_NP), "wke": wke.astype(BF_NP),
        "w2g": w2g, "gfc": gfc, "gfin": gfin, "whh": whh.astype(BF_NP),
        "wihb": wihb.astype(BF_NP),
        "b1": b1.astype(f32),
        "wfh": wfh.astype(BF_NP),
        "ident": np.eye(128, dtype=f32).astype(BF_NP),
    }

    x_full = np.ascontiguousarray(np.asarray(inputs["input_encoded"], f32))
    yh_full = np.asarray(inputs["y_history"], f32)[:, :, 0]           # [B_FULL, 127]

    in_maps = []
    for i in range(NCORES):
        sl = slice(i * B, (i + 1) * B)
        m = dict(weights)
        m["x"] = x_full[sl]
        m["yh"] = np.ascontiguousarray(yh_full[sl].T).astype(BF_NP)   # [127, B]
        in_maps.append(m)
    return in_maps, fc_wy, float(fcf_b[0])


_RUN_KW = {}


def _kernel_impl(inputs, n_steps):
    in_maps, fc_wy, fcf_b = _prep_host(inputs, n_steps)
    nc = _build(fc_wy, fcf_b, n_steps)
    res = run_bass_kernel_spmd(nc, in_maps, core_ids=list(range(NCORES)),
                               **_RUN_KW)
    out = np.concatenate(
        [np.asarray(res.results[i]["out"], np.float32).reshape(B, 1)
         for i in range(NCORES)], axis=0)
    return out, res


def kernel(**inputs) -> np.ndarray:
    out, _ = _kernel_impl(inputs, TSTEPS)
    return out


# revision 20
# speedup vs baseline: 1.0966x; 1.0966x over previous
"""Trainium2 Bass kernel for an attention-decoder LSTM (nn_Decoder).

Data-parallel over 8 NeuronCores: batch 4096 -> 512 per core. All weights
replicated. The T-1=127 step recurrence runs fully on-chip.

v2: DUAL-STREAM. The per-core batch (512) is split into two independent
streams of 256. The ACT (Scalar) engine is the hard bottleneck (tanh over
[E, T, Bh] every step has no alternative engine), so the two streams are
phase-shifted: while stream A runs its serial softmax/LSTM tail (~11us of
dependency chain), stream B's tanh chunks keep ACT busy. Tail instructions
are hand-interleaved between the other stream's chunk instructions so the
in-order ACT queue never stalls on a dependency.

Per stream and step:
  hp   = 0.5*W1_h.T @ H + 0.5*W1_c.T @ C          (PE, H=2h, C=2c)
  X    = tanh(enc_proj + hp)                       (DVE add + ACT tanh)
  e    = w2.T @ X      -> PSUM rows [t, b]         (PE, one-hot shift stationary)
  S    = exp(e)                                    (ACT)
  den  = ones.T @ S ; num = ones.T @ (S*pfc)       (PE)
  r    = num * recip_fast(den)  -> row0 of [r; yfc; 1] moving tile
  gates= s_g*(0.5*W_hh.T @ H) [issued early] + s_g*(W_ih (x) (r + yfc) + b)
  two fused tanh over [D, 2Bh] PSUM, LSTM update in tanh-only form
e rows and hp share one PSUM bank per stream (disjoint column ranges).
Final output row: 0.5*Wfh.T @ H + (ones.T @ (S*pfin))/den + fc_final_b.
"""

import numpy as np
import ml_dtypes

import concourse.bass as bass
import concourse.bacc as bacc
import concourse.tile as tile
from concourse import mybir
from concourse.bass_utils import run_bass_kernel_spmd

NCORES = 8
B_FULL, T, E, D = 4096, 128, 128, 128
B = B_FULL // NCORES        # 512 batch per core
BH = B // 2                 # 256 per stream
TSTEPS = T - 1              # 127

FP = mybir.dt.float32
BF = mybir.dt.bfloat16
AF = mybir.ActivationFunctionType
OP = mybir.AluOpType
BF_NP = ml_dtypes.bfloat16

TCH = 12                    # max t-chunk
CHUNKS = []
_t0 = 0
for _sz in [4] + [12] * 10 + [4]:
    CHUNKS.append((_t0, _sz))
    _t0 += _sz
assert _t0 == T
NCH = len(CHUNKS)


def _build(fc_final_b: float, n_steps: int):
    nc = bacc.Bacc("TRN2", target_bir_lowering=False, debug=False,
                   num_devices=NCORES)

    x_ext = nc.declare_dram_parameter("x", [B, T, E], FP, isOutput=False)
    # yfc = fc_wy * y_history, transposed to [TSTEPS, B]
    yfc_ext = nc.declare_dram_parameter("yfc", [max(TSTEPS, 1), B], BF,
                                        isOutput=False)
    w1hc_ext = nc.declare_dram_parameter("w1hc", [D, 2 * E], BF, isOutput=False)
    wke_ext = nc.declare_dram_parameter("wke", [E, E], BF, isOutput=False)
    w2g_ext = nc.declare_dram_parameter("w2g", [E, 2 * T], BF, isOutput=False)
    gfc_ext = nc.declare_dram_parameter("gfc", [E, 2 * T], BF, isOutput=False)
    gfin_ext = nc.declare_dram_parameter("gfin", [E, 2 * T], BF, isOutput=False)
    whh_ext = nc.declare_dram_parameter("whh", [D, 4 * D], BF, isOutput=False)
    # rows: [s_g*W_ih (x r), s_g*W_ih (x yfc), s_g*bias (x 1)]
    wihb_ext = nc.declare_dram_parameter("wihb", [3, 4 * D], BF, isOutput=False)
    b1_ext = nc.declare_dram_parameter("b1", [E, 1], FP, isOutput=False)
    wfh_ext = nc.declare_dram_parameter("wfh", [D, 1], BF, isOutput=False)
    id_ext = nc.declare_dram_parameter("ident", [128, 128], BF, isOutput=False)
    out_ext = nc.declare_dram_parameter("out", [1, B], FP, isOutput=True)

    with tile.TileContext(nc) as tc:
        import contextlib
        _stack = contextlib.ExitStack()
        const = _stack.enter_context(tc.tile_pool(name="const", bufs=1))
        state = _stack.enter_context(tc.tile_pool(name="state", bufs=2))
        xpool = _stack.enter_context(tc.tile_pool(name="xpool", bufs=3))
        tw = _stack.enter_context(tc.tile_pool(name="tw", bufs=1))
        intp = _stack.enter_context(tc.tile_pool(name="intp", bufs=2))
        dma4 = _stack.enter_context(tc.tile_pool(name="dma4", bufs=4))
        # PSUM: eh0+eh1 (2) + gA/gB x2 streams (4) + den+num (2) = 8 banks
        pseh = _stack.enter_context(tc.tile_pool(name="pseh", bufs=1, space="PSUM"))
        psg = _stack.enter_context(tc.tile_pool(name="psg", bufs=1, space="PSUM"))
        psm = _stack.enter_context(tc.tile_pool(name="psm", bufs=1, space="PSUM"))

        # ---- constants -------------------------------------------------
        w1hc_sb = const.tile([D, 2 * E], BF, tag="w1hc")
        nc.sync.dma_start(out=w1hc_sb[:], in_=w1hc_ext[:])
        wke_sb = const.tile([E, E], BF, tag="wke")
        nc.sync.dma_start(out=wke_sb[:], in_=wke_ext[:])
        w2g_sb = const.tile([E, 2 * T], BF, tag="w2g")
        nc.sync.dma_start(out=w2g_sb[:], in_=w2g_ext[:])
        gfc_sb = const.tile([E, 2 * T], BF, tag="gfc")
        nc.sync.dma_start(out=gfc_sb[:], in_=gfc_ext[:])
        gfin_sb = const.tile([E, 2 * T], BF, tag="gfin")
        nc.sync.dma_start(out=gfin_sb[:], in_=gfin_ext[:])
        whh_sb = const.tile([D, 4 * D], BF, tag="whh")
        nc.sync.dma_start(out=whh_sb[:], in_=whh_ext[:])
        wihb_sb = const.tile([3, 4 * D], BF, tag="wihb")
        nc.sync.dma_start(out=wihb_sb[:], in_=wihb_ext[:])
        b1_sb = const.tile([E, 1], FP, tag="b1")
        nc.sync.dma_start(out=b1_sb[:], in_=b1_ext[:])
        wfh_sb = const.tile([D, 1], BF, tag="wfh")
        nc.sync.dma_start(out=wfh_sb[:], in_=wfh_ext[:])
        id_sb = const.tile([128, 128], BF, tag="ident")
        nc.sync.dma_start(out=id_sb[:], in_=id_ext[:])
        ones_sb = const.tile([T, 1], BF, tag="ones")
        nc.vector.memset(ones_sb[:], 1.0)
        # per-stream moving operand [r; yfc; 1] for the W_ih/bias gate matmul
        ytr = []
        for h in range(2):
            yt_h = const.tile([3, BH], BF, tag=f"ytr{h}")
            nc.vector.memset(yt_h[:], 1.0)
            ytr.append(yt_h)

        encp = const.tile([E, T, B], BF, tag="encp")
        pfc_sb = const.tile([T, B], BF, tag="pfc")
        pfin_sb = const.tile([T, B], BF, tag="pfin")
        C = const.tile([D, B], FP, tag="C")   # 2*c, streams side by side
        nc.vector.memset(C[:], 0.0)

        # ---- precompute: enc_proj, pfc, pfin ---------------------------
        pfc_ps = psm.tile([T, B], FP, tag="den")
        pfin_ps = psm.tile([T, B], FP, tag="num")
        for t in range(T):
            inT_ps = psg.tile([E, B], BF, tag="gA0")
            for blk in range(B // 128):
                xin = dma4.tile([128, E], FP, tag="xin")
                nc.sync.dma_start(
                    out=xin[:],
                    in_=x_ext[blk * 128:(blk + 1) * 128, t, :],
                )
                xbf = dma4.tile([128, E], BF, tag="xbf")
                nc.vector.tensor_copy(xbf[:], xin[:])
                nc.tensor.transpose(
                    inT_ps[:, blk * 128:(blk + 1) * 128], xbf[:], id_sb[:]
                )
            inT = intp.tile([E, B], BF, tag="inT")
            nc.vector.tensor_copy(inT[:], inT_ps[:])
            ep_ps = psg.tile([E, B], FP, tag="gB0")
            nc.tensor.matmul(ep_ps[:], wke_sb[:], inT[:],
                             start=True, stop=True)
            nc.tensor.matmul(pfc_ps[:], gfc_sb[:, T - 1 - t:2 * T - 1 - t],
                             inT[:], start=(t == 0), stop=(t == T - 1))
            nc.tensor.matmul(pfin_ps[:], gfin_sb[:, T - 1 - t:2 * T - 1 - t],
                             inT[:], start=(t == 0), stop=(t == T - 1))
            nc.scalar.activation(encp[:, t, :], ep_ps[:],
                                 AF.Identity, bias=b1_sb[:], scale=1.0)
        nc.vector.tensor_copy(pfc_sb[:], pfc_ps[:])
        nc.vector.tensor_copy(pfin_sb[:], pfin_ps[:])

        # ---- per-stream mutable handles -------------------------------
        Hbf = [None, None]
        Cbf = [None, None]
        hp_sb = [None, None]
        eh = [None, None]     # [128, 2*BH] psum: cols 0:BH = e rows, BH:2BH = hp
        gA = [None, None]     # [D, 2*BH] psum: [i | f] gate preacts
        gB = [None, None]     # [D, 2*BH] psum: [g | o]
        S_sb = [None, None]
        rcp = [None, None]
        for h in range(2):
            Hbf[h] = state.tile([D, BH], BF, tag=f"Hbf{h}", name=f"Hbf{h}")
            nc.vector.memset(Hbf[h][:], 0.0)
            Cbf[h] = state.tile([D, BH], BF, tag=f"Cbf{h}", name=f"Cbf{h}")
            nc.vector.memset(Cbf[h][:], 0.0)
            hp_sb[h] = state.tile([E, BH], BF, tag=f"hp{h}", name=f"hp{h}")
            nc.vector.memset(hp_sb[h][:], 0.0)
            nc.sync.dma_start(out=ytr[h][1:2, :],
                              in_=yfc_ext[0:1, h * BH:(h + 1) * BH])
            eh[h] = pseh.tile([128, 2 * BH], FP, tag=f"eh{h}", name=f"eh{h}")

        def emit_chunk(h, ci):
            t0, tcsz = CHUNKS[ci]
            csl = slice(h * BH, (h + 1) * BH)
            X = xpool.tile([E, TCH, BH], BF, tag=f"X{h}")
            hp_b = hp_sb[h][:].unsqueeze(1).broadcast_to([E, tcsz, BH])
            nc.vector.tensor_tensor(X[:, :tcsz, :], encp[:, t0:t0 + tcsz, csl],
                                    hp_b, op=OP.add)
            nc.scalar.activation(X[:, :tcsz, :], X[:, :tcsz, :], AF.Tanh)
            for j in range(tcsz):
                t = t0 + j
                nc.tensor.matmul(eh[h][:, 0:BH],
                                 w2g_sb[:, T - 1 - t:2 * T - 1 - t],
                                 X[:, j, :], start=(t == 0), stop=(t == T - 1))

        def tail_soft(h, s):
            # softmax-weighted average r = sum(S*pfc)/sum(S); writes ytr row0
            csl = slice(h * BH, (h + 1) * BH)
            S_sb[h] = tw.tile([T, BH], BF, tag=f"S{h}", name=f"S{h}")
            nc.scalar.activation(S_sb[h][:], eh[h][:, 0:BH], AF.Exp)
            den_ps = psm.tile([1, BH], FP, tag="den")
            nc.tensor.matmul(den_ps[:], ones_sb[:], S_sb[h][:],
                             start=True, stop=True)
            SP = tw.tile([T, BH], BF, tag=f"SP{h}")
            nc.vector.tensor_tensor(SP[:], S_sb[h][:], pfc_sb[:, csl],
                                    op=OP.mult)
            num_ps = psm.tile([1, BH], FP, tag="num")
            nc.tensor.matmul(num_ps[:], ones_sb[:], SP[:],
                             start=True, stop=True)
            rcp[h] = tw.tile([1, BH], FP, tag=f"rcp{h}", name=f"rcp{h}")
            nc.vector.reciprocal_approx_fast(rcp[h][:], den_ps[:])
            nc.vector.tensor_tensor(ytr[h][0:1, :], num_ps[:], rcp[h][:],
                                    op=OP.mult)

        def tail_gates(h):
            # per gate: one closed accumulation group (W_hh then W_ih/bias,
            # back to back) -- never two interleaved open groups in one bank
            gA[h] = psg.tile([D, 2 * BH], FP, tag=f"gA{h}", name=f"gA{h}")
            gB[h] = psg.tile([D, 2 * BH], FP, tag=f"gB{h}", name=f"gB{h}")
            for g in range(4):
                gt = gA[h] if g < 2 else gB[h]
                off = (g % 2) * BH
                nc.tensor.matmul(gt[:, off:off + BH],
                                 whh_sb[:, g * D:(g + 1) * D], Hbf[h][:],
                                 start=True, stop=False)
                nc.tensor.matmul(gt[:, off:off + BH],
                                 wihb_sb[:, g * D:(g + 1) * D], ytr[h][:],
                                 start=False, stop=True)
            tgA = tw.tile([D, 2 * BH], FP, tag=f"tgA{h}")
            nc.scalar.activation(tgA[:], gA[h][:], AF.Tanh)
            tgB = tw.tile([D, 2 * BH], FP, tag=f"tgB{h}")
            nc.scalar.activation(tgB[:], gB[h][:], AF.Tanh)
            return tgA, tgB

        def tail_lstm(h, tgA, tgB):
            csl = slice(h * BH, (h + 1) * BH)
            tmp1 = tw.tile([D, BH], FP, tag=f"tmp1{h}")
            nc.vector.scalar_tensor_tensor(tmp1[:], tgA[:, BH:2 * BH], 1.0,
                                           C[:, csl], op0=OP.add, op1=OP.mult)
            tmp2 = tw.tile([D, BH], FP, tag=f"tmp2{h}")
            nc.vector.scalar_tensor_tensor(tmp2[:], tgA[:, 0:BH], 1.0,
                                           tgB[:, 0:BH], op0=OP.add, op1=OP.mult)
            nc.vector.scalar_tensor_tensor(C[:, csl], tmp1[:], 0.5, tmp2[:],
                                           op0=OP.mult, op1=OP.add)
            tct = tw.tile([D, BH], FP, tag=f"tct{h}")
            nc.scalar.activation(tct[:], C[:, csl], AF.Tanh, scale=0.5)
            Hbf[h] = state.tile([D, BH], BF, tag=f"Hbf{h}", name=f"Hbf{h}")
            nc.vector.scalar_tensor_tensor(Hbf[h][:], tgB[:, BH:2 * BH], 1.0,
                                           tct[:], op0=OP.add, op1=OP.mult)
            Cbf[h] = state.tile([D, BH], BF, tag=f"Cbf{h}", name=f"Cbf{h}")
            nc.vector.tensor_copy(Cbf[h][:], C[:, csl])

        def tail_next(h, s):
            # hp for step s+1 + early W_hh gate matmuls + yfc prefetch
            eh[h] = pseh.tile([128, 2 * BH], FP, tag=f"eh{h}", name=f"eh{h}")
            hp_ps = eh[h][:, BH:2 * BH]
            nc.tensor.matmul(hp_ps, w1hc_sb[:, E:2 * E], Cbf[h][:],
                             start=True, stop=False)
            nc.tensor.matmul(hp_ps, w1hc_sb[:, 0:E], Hbf[h][:],
                             start=False, stop=True)
            hp_sb[h] = state.tile([E, BH], BF, tag=f"hp{h}", name=f"hp{h}")
            nc.vector.tensor_copy(hp_sb[h][:], hp_ps)
            nc.sync.dma_start(out=ytr[h][1:2, :],
                              in_=yfc_ext[s + 1:s + 2, h * BH:(h + 1) * BH])

        def emit_tail(h, s, chunk_emitter):
            """Emit stream h's tail, interleaved with another stream's
            chunks via chunk_emitter(k) for pacing slots k=0..7."""
            chunk_emitter(0)
            chunk_emitter(1)
            tail_soft(h, s)
            chunk_emitter(2)
            chunk_emitter(3)
            tgA, tgB = tail_gates(h)
            chunk_emitter(4)
            chunk_emitter(5)
            tail_lstm(h, tgA, tgB)
            chunk_emitter(6)
            chunk_emitter(7)
            if s + 1 < n_steps:
                tail_next(h, s)
            for k in range(8, NCH):
                chunk_emitter(k)

        # ---- the recurrence -------------------------------------------
        # stream 0's X phase for step 0
        for ci in range(NCH):
            emit_chunk(0, ci)
        import os
        _mode = int(os.environ.get("V2_PACE_MODE", "0"))
        for s in range(n_steps):
            if _mode == 2:
                # fully sequential (debug): no interleaving at all
                for ci in range(NCH):
                    emit_chunk(1, ci)
                emit_tail(0, s, lambda k: None)
                emit_tail(1, s, lambda k: None)
                if s + 1 < n_steps:
                    for ci in range(NCH):
                        emit_chunk(0, ci)
                continue
            # stream 1 X phase (step s) hides stream 0's tail (step s)
            emit_tail(0, s, lambda k: emit_chunk(1, k))
            if s + 1 < n_steps and _mode == 0:
                # stream 0 X phase (step s+1) hides stream 1's tail (step s)
                emit_tail(1, s, lambda k: emit_chunk(0, k))
            else:
                emit_tail(1, s, lambda k: None)
                if s + 1 < n_steps:
                    for ci in range(NCH):
                        emit_chunk(0, ci)

        # ---- final output row ----------------------------------------
        for h in range(2):
            csl = slice(h * BH, (h + 1) * BH)
            o_ps = psm.tile([1, BH], FP, tag="den")
            nc.tensor.matmul(o_ps[:], wfh_sb[:], Hbf[h][:],
                             start=True, stop=True)
            o_sb = tw.tile([1, BH], FP, tag=f"osb{h}")
            if n_steps > 0:
                SPf = tw.tile([T, BH], BF, tag=f"SP{h}")
                nc.vector.tensor_tensor(SPf[:], S_sb[h][:], pfin_sb[:, csl],
                                        op=OP.mult)
                nf_ps = psm.tile([1, BH], FP, tag="num")
                nc.tensor.matmul(nf_ps[:], ones_sb[:], SPf[:],
                                 start=True, stop=True)
                rfin = tw.tile([1, BH], FP, tag=f"rfin{h}")
                nc.vector.tensor_tensor(rfin[:], nf_ps[:], rcp[h][:],
                                        op=OP.mult)
                nc.vector.scalar_tensor_tensor(o_sb[:], o_ps[:], fc_final_b,
                                               rfin[:], op0=OP.add, op1=OP.add)
            else:
                nc.vector.tensor_scalar_add(o_sb[:], o_ps[:], fc_final_b)
            nc.sync.dma_start(out=out_ext[0:1, csl], in_=o_sb[:])
        _stack.close()

    nc.finalize()
    return nc


def _prep_host(inputs, n_steps):
    f32 = np.float32
    attn_W1 = np.asarray(inputs["attn_W1"], f32)
    attn_W2 = np.asarray(inputs["attn_W2"], f32)
    W_ih = np.asarray(inputs["W_ih"], f32)
    W_hh = np.asarray(inputs["W_hh"], f32)
    b_ih = np.asarray(inputs["b_ih"], f32)
    b_hh = np.asarray(inputs["b_hh"], f32)
    fc_W = np.asarray(inputs["fc_W"], f32)
    fc_b = np.asarray(inputs["fc_b"], f32)
    fcf_W = np.asarray(inputs["fc_final_W"], f32)
    fcf_b = np.asarray(inputs["fc_final_b"], f32)

    W1_h = attn_W1[:, :D]
    W1_c = attn_W1[:, D:2 * D]
    W1_e = attn_W1[:, 2 * D:]

    w1hc = np.concatenate([0.5 * W1_h.T, 0.5 * W1_c.T], axis=1)      # [D, 2E]
    wke = np.ascontiguousarray(W1_e.T)                                # [E, E]
    def onehot_shift(vec):
        g = np.zeros((E, 2 * T), f32)
        g[:, T - 1] = vec
        return g.astype(BF_NP)
    w2g = onehot_shift(attn_W2[0])
    gfc = onehot_shift(fc_W[0, :E])
    gfin = onehot_shift(fcf_W[0, D:])
    fc_wy = float(fc_W[0, E])
    wfh = 0.5 * fcf_W[0, :D][:, None]                                 # [D, 1]

    scales = np.array([0.5, 0.5, 1.0, 0.5], f32)
    gate_scale = np.repeat(scales, D)                                 # [4D]
    whh = (0.5 * W_hh.T) * gate_scale[None, :]                        # [D, 4D]
    bs = b_ih + b_hh + W_ih[:, 0] * float(fc_b[0])                    # [4D]
    wihb = np.stack([W_ih[:, 0] * gate_scale,
                     W_ih[:, 0] * gate_scale,
                     bs * gate_scale], axis=0)                        # [3, 4D]
    b1 = np.asarray(inputs["attn_b1"], f32)[:, None]

    weights = {
        "w1hc": w1hc.astype(BF_NP), "wke": wke.astype(BF_NP),
        "w2g": w2g, "gfc": gfc, "gfin": gfin, "whh": whh.astype(BF_NP),
        "wihb": wihb.astype(BF_NP),
        "b1": b1.astype(f32),
        "wfh": wfh.astype(BF_NP),
        "ident": np.eye(128, dtype=f32).astype(BF_NP),
    }

    x_full = np.ascontiguousarray(np.asarray(inputs["input_encoded"], f32))
    yh_full = np.asarray(inputs["y_history"], f32)[:, :, 0]           # [B_FULL, 127]

    in_maps = []
    for i in range(NCORES):
        sl = slice(i * B, (i + 1) * B)
        m = dict(weights)
        m["x"] = x_full[sl]
        m["yfc"] = np.ascontiguousarray(
            (fc_wy * yh_full[sl]).T).astype(BF_NP)                    # [127, B]
        in_maps.append(m)
    return in_maps, float(fcf_b[0])


_RUN_KW = {}


def _kernel_impl(inputs, n_steps):
    in_maps, fcf_b = _prep_host(inputs, n_steps)
    nc = _build(fcf_b, n_steps)
    res = run_bass_kernel_spmd(nc, in_maps, core_ids=list(range(NCORES)),
                               **_RUN_KW)
    out = np.concatenate(
        [np.asarray(res.results[i]["out"], np.float32).reshape(B, 1)
         for i in range(NCORES)], axis=0)
    return out, res


def kernel(**inputs) -> np.ndarray:
    out, _ = _kernel_impl(inputs, TSTEPS)
    return out


# revision 21
# speedup vs baseline: 1.1455x; 1.0446x over previous
"""Trainium2 Bass kernel for an attention-decoder LSTM (nn_Decoder).

Data-parallel over 8 NeuronCores: batch 4096 -> 512 per core. All weights
replicated. The T-1=127 step recurrence runs fully on-chip.

v2: DUAL-STREAM. The per-core batch (512) is split into two independent
streams of 256. The ACT (Scalar) engine is the hard bottleneck (tanh over
[E, T, Bh] every step has no alternative engine), so the two streams are
phase-shifted: while stream A runs its serial softmax/LSTM tail (~11us of
dependency chain), stream B's tanh chunks keep ACT busy. Tail instructions
are hand-interleaved between the other stream's chunk instructions so the
in-order ACT queue never stalls on a dependency.

Per stream and step:
  hp   = 0.5*W1_h.T @ H + 0.5*W1_c.T @ C          (PE, H=2h, C=2c)
  X    = tanh(enc_proj + hp)                       (DVE add + ACT tanh)
  e    = w2.T @ X      -> PSUM rows [t, b]         (PE, one-hot shift stationary)
  S    = exp(e)                                    (ACT)
  den  = ones.T @ S ; num = ones.T @ (S*pfc)       (PE)
  r    = num * recip_fast(den)  -> row0 of [r; yfc; 1] moving tile
  gates= s_g*(0.5*W_hh.T @ H) [issued early] + s_g*(W_ih (x) (r + yfc) + b)
  two fused tanh over [D, 2Bh] PSUM, LSTM update in tanh-only form
e rows and hp share one PSUM bank per stream (disjoint column ranges).
Final output row: 0.5*Wfh.T @ H + (ones.T @ (S*pfin))/den + fc_final_b.
"""

import numpy as np
import ml_dtypes

import concourse.bass as bass
import concourse.bacc as bacc
import concourse.tile as tile
from concourse import mybir
from concourse.bass_utils import run_bass_kernel_spmd

NCORES = 8
B_FULL, T, E, D = 4096, 128, 128, 128
B = B_FULL // NCORES        # 512 batch per core
BH = B // 2                 # 256 per stream
TSTEPS = T - 1              # 127

FP = mybir.dt.float32
BF = mybir.dt.bfloat16
AF = mybir.ActivationFunctionType
OP = mybir.AluOpType
BF_NP = ml_dtypes.bfloat16

TCH = 16                    # t-chunk (restart latency is hidden by the
CHUNKS = [(i * 16, 16) for i in range(8)]   # other stream, so no taper)
NCH = len(CHUNKS)


def _build(fc_final_b: float, n_steps: int):
    nc = bacc.Bacc("TRN2", target_bir_lowering=False, debug=False,
                   num_devices=NCORES)

    x_ext = nc.declare_dram_parameter("x", [B, T, E], FP, isOutput=False)
    # yfc = fc_wy * y_history, transposed to [TSTEPS, B]
    yfc_ext = nc.declare_dram_parameter("yfc", [max(TSTEPS, 1), B], BF,
                                        isOutput=False)
    w1hc_ext = nc.declare_dram_parameter("w1hc", [D, 2 * E], BF, isOutput=False)
    wke_ext = nc.declare_dram_parameter("wke", [E, E], BF, isOutput=False)
    w2g_ext = nc.declare_dram_parameter("w2g", [E, 2 * T], BF, isOutput=False)
    gfc_ext = nc.declare_dram_parameter("gfc", [E, 2 * T], BF, isOutput=False)
    gfin_ext = nc.declare_dram_parameter("gfin", [E, 2 * T], BF, isOutput=False)
    whh_ext = nc.declare_dram_parameter("whh", [D, 4 * D], BF, isOutput=False)
    # rows: [s_g*W_ih (x r), s_g*W_ih (x yfc), s_g*bias (x 1)]
    wihb_ext = nc.declare_dram_parameter("wihb", [3, 4 * D], BF, isOutput=False)
    b1_ext = nc.declare_dram_parameter("b1", [E, 1], FP, isOutput=False)
    wfh_ext = nc.declare_dram_parameter("wfh", [D, 1], BF, isOutput=False)
    id_ext = nc.declare_dram_parameter("ident", [128, 128], BF, isOutput=False)
    out_ext = nc.declare_dram_parameter("out", [1, B], FP, isOutput=True)

    with tile.TileContext(nc) as tc:
        import contextlib
        _stack = contextlib.ExitStack()
        const = _stack.enter_context(tc.tile_pool(name="const", bufs=1))
        state = _stack.enter_context(tc.tile_pool(name="state", bufs=2))
        xpool = _stack.enter_context(tc.tile_pool(name="xpool", bufs=4))
        tw = _stack.enter_context(tc.tile_pool(name="tw", bufs=1))
        intp = _stack.enter_context(tc.tile_pool(name="intp", bufs=2))
        dma4 = _stack.enter_context(tc.tile_pool(name="dma4", bufs=4))
        # PSUM: eh0+eh1 (2) + gA/gB x2 streams (4) + den+num (2) = 8 banks
        pseh = _stack.enter_context(tc.tile_pool(name="pseh", bufs=1, space="PSUM"))
        psg = _stack.enter_context(tc.tile_pool(name="psg", bufs=1, space="PSUM"))
        psm = _stack.enter_context(tc.tile_pool(name="psm", bufs=1, space="PSUM"))

        # ---- constants -------------------------------------------------
        w1hc_sb = const.tile([D, 2 * E], BF, tag="w1hc")
        nc.sync.dma_start(out=w1hc_sb[:], in_=w1hc_ext[:])
        wke_sb = const.tile([E, E], BF, tag="wke")
        nc.sync.dma_start(out=wke_sb[:], in_=wke_ext[:])
        w2g_sb = const.tile([E, 2 * T], BF, tag="w2g")
        nc.sync.dma_start(out=w2g_sb[:], in_=w2g_ext[:])
        gfc_sb = const.tile([E, 2 * T], BF, tag="gfc")
        nc.sync.dma_start(out=gfc_sb[:], in_=gfc_ext[:])
        gfin_sb = const.tile([E, 2 * T], BF, tag="gfin")
        nc.sync.dma_start(out=gfin_sb[:], in_=gfin_ext[:])
        whh_sb = const.tile([D, 4 * D], BF, tag="whh")
        nc.sync.dma_start(out=whh_sb[:], in_=whh_ext[:])
        wihb_sb = const.tile([3, 4 * D], BF, tag="wihb")
        nc.sync.dma_start(out=wihb_sb[:], in_=wihb_ext[:])
        b1_sb = const.tile([E, 1], FP, tag="b1")
        nc.sync.dma_start(out=b1_sb[:], in_=b1_ext[:])
        wfh_sb = const.tile([D, 1], BF, tag="wfh")
        nc.sync.dma_start(out=wfh_sb[:], in_=wfh_ext[:])
        id_sb = const.tile([128, 128], BF, tag="ident")
        nc.sync.dma_start(out=id_sb[:], in_=id_ext[:])
        ones_sb = const.tile([T, 1], BF, tag="ones")
        nc.vector.memset(ones_sb[:], 1.0)
        # per-stream moving operand [r; yfc; 1] for the W_ih/bias gate matmul
        ytr = []
        for h in range(2):
            yt_h = const.tile([3, BH], BF, tag=f"ytr{h}")
            nc.vector.memset(yt_h[:], 1.0)
            ytr.append(yt_h)

        encp = const.tile([E, T, B], BF, tag="encp")
        pfc_sb = const.tile([T, B], BF, tag="pfc")
        pfin_sb = const.tile([T, B], BF, tag="pfin")
        C = const.tile([D, B], FP, tag="C")   # 2*c, streams side by side
        nc.vector.memset(C[:], 0.0)

        # ---- precompute: enc_proj, pfc, pfin ---------------------------
        pfc_ps = psm.tile([T, B], FP, tag="den")
        pfin_ps = psm.tile([T, B], FP, tag="num")
        for t in range(T):
            inT_ps = psg.tile([E, B], BF, tag="gA0")
            for blk in range(B // 128):
                xin = dma4.tile([128, E], FP, tag="xin")
                nc.sync.dma_start(
                    out=xin[:],
                    in_=x_ext[blk * 128:(blk + 1) * 128, t, :],
                )
                xbf = dma4.tile([128, E], BF, tag="xbf")
                nc.vector.tensor_copy(xbf[:], xin[:])
                nc.tensor.transpose(
                    inT_ps[:, blk * 128:(blk + 1) * 128], xbf[:], id_sb[:]
                )
            inT = intp.tile([E, B], BF, tag="inT")
            nc.vector.tensor_copy(inT[:], inT_ps[:])
            ep_ps = psg.tile([E, B], FP, tag="gB0")
            nc.tensor.matmul(ep_ps[:], wke_sb[:], inT[:],
                             start=True, stop=True)
            nc.tensor.matmul(pfc_ps[:], gfc_sb[:, T - 1 - t:2 * T - 1 - t],
                             inT[:], start=(t == 0), stop=(t == T - 1))
            nc.tensor.matmul(pfin_ps[:], gfin_sb[:, T - 1 - t:2 * T - 1 - t],
                             inT[:], start=(t == 0), stop=(t == T - 1))
            nc.scalar.activation(encp[:, t, :], ep_ps[:],
                                 AF.Identity, bias=b1_sb[:], scale=1.0)
        nc.vector.tensor_copy(pfc_sb[:], pfc_ps[:])
        nc.vector.tensor_copy(pfin_sb[:], pfin_ps[:])

        # ---- per-stream mutable handles -------------------------------
        Hbf = [None, None]
        Cbf = [None, None]
        hp_sb = [None, None]
        eh = [None, None]     # [128, 2*BH] psum: cols 0:BH = e rows, BH:2BH = hp
        gA = [None, None]     # [D, 2*BH] psum: [i | f] gate preacts
        gB = [None, None]     # [D, 2*BH] psum: [g | o]
        S_sb = [None, None]
        rcp = [None, None]
        for h in range(2):
            Hbf[h] = state.tile([D, BH], BF, tag=f"Hbf{h}", name=f"Hbf{h}")
            nc.vector.memset(Hbf[h][:], 0.0)
            Cbf[h] = state.tile([D, BH], BF, tag=f"Cbf{h}", name=f"Cbf{h}")
            nc.vector.memset(Cbf[h][:], 0.0)
            hp_sb[h] = state.tile([E, BH], BF, tag=f"hp{h}", name=f"hp{h}")
            nc.vector.memset(hp_sb[h][:], 0.0)
            nc.sync.dma_start(out=ytr[h][1:2, :],
                              in_=yfc_ext[0:1, h * BH:(h + 1) * BH])
            eh[h] = pseh.tile([128, 2 * BH], FP, tag=f"eh{h}", name=f"eh{h}")

        def emit_chunk(h, ci):
            t0, tcsz = CHUNKS[ci]
            csl = slice(h * BH, (h + 1) * BH)
            X = xpool.tile([E, TCH, BH], BF, tag="X", name="X")
            hp_b = hp_sb[h][:].unsqueeze(1).broadcast_to([E, tcsz, BH])
            nc.vector.tensor_tensor(X[:, :tcsz, :], encp[:, t0:t0 + tcsz, csl],
                                    hp_b, op=OP.add)
            nc.scalar.activation(X[:, :tcsz, :], X[:, :tcsz, :], AF.Tanh)
            for j in range(tcsz):
                t = t0 + j
                nc.tensor.matmul(eh[h][:, 0:BH],
                                 w2g_sb[:, T - 1 - t:2 * T - 1 - t],
                                 X[:, j, :], start=(t == 0), stop=(t == T - 1))

        def tail_soft(h, s):
            # softmax-weighted average r = sum(S*pfc)/sum(S); writes ytr row0
            csl = slice(h * BH, (h + 1) * BH)
            S_sb[h] = tw.tile([T, BH], BF, tag=f"S{h}", name=f"S{h}")
            nc.scalar.activation(S_sb[h][:], eh[h][:, 0:BH], AF.Exp)
            den_ps = psm.tile([1, BH], FP, tag="den")
            nc.tensor.matmul(den_ps[:], ones_sb[:], S_sb[h][:],
                             start=True, stop=True)
            SP = tw.tile([T, BH], BF, tag=f"SP{h}")
            nc.vector.tensor_tensor(SP[:], S_sb[h][:], pfc_sb[:, csl],
                                    op=OP.mult)
            num_ps = psm.tile([1, BH], FP, tag="num")
            nc.tensor.matmul(num_ps[:], ones_sb[:], SP[:],
                             start=True, stop=True)
            rcp[h] = tw.tile([1, BH], FP, tag=f"rcp{h}", name=f"rcp{h}")
            nc.vector.reciprocal_approx_fast(rcp[h][:], den_ps[:])
            nc.vector.tensor_tensor(ytr[h][0:1, :], num_ps[:], rcp[h][:],
                                    op=OP.mult)

        def tail_gates(h):
            # per gate: one closed accumulation group (W_hh then W_ih/bias,
            # back to back) -- never two interleaved open groups in one bank
            gA[h] = psg.tile([D, 2 * BH], FP, tag=f"gA{h}", name=f"gA{h}")
            gB[h] = psg.tile([D, 2 * BH], FP, tag=f"gB{h}", name=f"gB{h}")
            for g in range(4):
                gt = gA[h] if g < 2 else gB[h]
                off = (g % 2) * BH
                nc.tensor.matmul(gt[:, off:off + BH],
                                 whh_sb[:, g * D:(g + 1) * D], Hbf[h][:],
                                 start=True, stop=False)
                nc.tensor.matmul(gt[:, off:off + BH],
                                 wihb_sb[:, g * D:(g + 1) * D], ytr[h][:],
                                 start=False, stop=True)
            tgA = tw.tile([D, 2 * BH], FP, tag=f"tgA{h}")
            nc.scalar.activation(tgA[:], gA[h][:], AF.Tanh)
            tgB = tw.tile([D, 2 * BH], FP, tag=f"tgB{h}")
            nc.scalar.activation(tgB[:], gB[h][:], AF.Tanh)
            return tgA, tgB

        def tail_lstm(h, tgA, tgB):
            csl = slice(h * BH, (h + 1) * BH)
            tmp1 = tw.tile([D, BH], FP, tag=f"tmp1{h}")
            nc.vector.scalar_tensor_tensor(tmp1[:], tgA[:, BH:2 * BH], 1.0,
                                           C[:, csl], op0=OP.add, op1=OP.mult)
            tmp2 = tw.tile([D, BH], FP, tag=f"tmp2{h}")
            nc.vector.scalar_tensor_tensor(tmp2[:], tgA[:, 0:BH], 1.0,
                                           tgB[:, 0:BH], op0=OP.add, op1=OP.mult)
            nc.vector.scalar_tensor_tensor(C[:, csl], tmp1[:], 0.5, tmp2[:],
                                           op0=OP.mult, op1=OP.add)
            tct = tw.tile([D, BH], FP, tag=f"tct{h}")
            nc.scalar.activation(tct[:], C[:, csl], AF.Tanh, scale=0.5)
            Hbf[h] = state.tile([D, BH], BF, tag=f"Hbf{h}", name=f"Hbf{h}")
            nc.vector.scalar_tensor_tensor(Hbf[h][:], tgB[:, BH:2 * BH], 1.0,
                                           tct[:], op0=OP.add, op1=OP.mult)
            Cbf[h] = state.tile([D, BH], BF, tag=f"Cbf{h}", name=f"Cbf{h}")
            nc.vector.tensor_copy(Cbf[h][:], C[:, csl])

        def tail_next(h, s):
            # hp for step s+1 + early W_hh gate matmuls + yfc prefetch
            eh[h] = pseh.tile([128, 2 * BH], FP, tag=f"eh{h}", name=f"eh{h}")
            hp_ps = eh[h][:, BH:2 * BH]
            nc.tensor.matmul(hp_ps, w1hc_sb[:, E:2 * E], Cbf[h][:],
                             start=True, stop=False)
            nc.tensor.matmul(hp_ps, w1hc_sb[:, 0:E], Hbf[h][:],
                             start=False, stop=True)
            hp_sb[h] = state.tile([E, BH], BF, tag=f"hp{h}", name=f"hp{h}")
            nc.vector.tensor_copy(hp_sb[h][:], hp_ps)
            nc.sync.dma_start(out=ytr[h][1:2, :],
                              in_=yfc_ext[s + 1:s + 2, h * BH:(h + 1) * BH])

        def emit_tail(h, s, chunk_emitter):
            """Emit stream h's tail, interleaved with another stream's
            chunks via chunk_emitter(k) for pacing slots k=0..7."""
            chunk_emitter(0)
            chunk_emitter(1)
            tail_soft(h, s)
            chunk_emitter(2)
            chunk_emitter(3)
            tgA, tgB = tail_gates(h)
            chunk_emitter(4)
            chunk_emitter(5)
            tail_lstm(h, tgA, tgB)
            chunk_emitter(6)
            chunk_emitter(7)
            if s + 1 < n_steps:
                tail_next(h, s)
            for k in range(8, NCH):
                chunk_emitter(k)

        # ---- the recurrence -------------------------------------------
        # stream 0's X phase for step 0
        for ci in range(NCH):
            emit_chunk(0, ci)
        import os
        _mode = int(os.environ.get("V2_PACE_MODE", "0"))
        for s in range(n_steps):
            if _mode == 2:
                # fully sequential (debug): no interleaving at all
                for ci in range(NCH):
                    emit_chunk(1, ci)
                emit_tail(0, s, lambda k: None)
                emit_tail(1, s, lambda k: None)
                if s + 1 < n_steps:
                    for ci in range(NCH):
                        emit_chunk(0, ci)
                continue
            # stream 1 X phase (step s) hides stream 0's tail (step s)
            emit_tail(0, s, lambda k: emit_chunk(1, k))
            if s + 1 < n_steps and _mode == 0:
                # stream 0 X phase (step s+1) hides stream 1's tail (step s)
                emit_tail(1, s, lambda k: emit_chunk(0, k))
            else:
                emit_tail(1, s, lambda k: None)
                if s + 1 < n_steps:
                    for ci in range(NCH):
                        emit_chunk(0, ci)

        # ---- final output row ----------------------------------------
        for h in range(2):
            csl = slice(h * BH, (h + 1) * BH)
            o_ps = psm.tile([1, BH], FP, tag="den")
            nc.tensor.matmul(o_ps[:], wfh_sb[:], Hbf[h][:],
                             start=True, stop=True)
            o_sb = tw.tile([1, BH], FP, tag=f"osb{h}")
            if n_steps > 0:
                SPf = tw.tile([T, BH], BF, tag=f"SP{h}")
                nc.vector.tensor_tensor(SPf[:], S_sb[h][:], pfin_sb[:, csl],
                                        op=OP.mult)
                nf_ps = psm.tile([1, BH], FP, tag="num")
                nc.tensor.matmul(nf_ps[:], ones_sb[:], SPf[:],
                                 start=True, stop=True)
                rfin = tw.tile([1, BH], FP, tag=f"rfin{h}")
                nc.vector.tensor_tensor(rfin[:], nf_ps[:], rcp[h][:],
                                        op=OP.mult)
                nc.vector.scalar_tensor_tensor(o_sb[:], o_ps[:], fc_final_b,
                                               rfin[:], op0=OP.add, op1=OP.add)
            else:
                nc.vector.tensor_scalar_add(o_sb[:], o_ps[:], fc_final_b)
            nc.sync.dma_start(out=out_ext[0:1, csl], in_=o_sb[:])
        _stack.close()

    nc.finalize()
    return nc


def _prep_host(inputs, n_steps):
    f32 = np.float32
    attn_W1 = np.asarray(inputs["attn_W1"], f32)
    attn_W2 = np.asarray(inputs["attn_W2"], f32)
    W_ih = np.asarray(inputs["W_ih"], f32)
    W_hh = np.asarray(inputs["W_hh"], f32)
    b_ih = np.asarray(inputs["b_ih"], f32)
    b_hh = np.asarray(inputs["b_hh"], f32)
    fc_W = np.asarray(inputs["fc_W"], f32)
    fc_b = np.asarray(inputs["fc_b"], f32)
    fcf_W = np.asarray(inputs["fc_final_W"], f32)
    fcf_b = np.asarray(inputs["fc_final_b"], f32)

    W1_h = attn_W1[:, :D]
    W1_c = attn_W1[:, D:2 * D]
    W1_e = attn_W1[:, 2 * D:]

    w1hc = np.concatenate([0.5 * W1_h.T, 0.5 * W1_c.T], axis=1)      # [D, 2E]
    wke = np.ascontiguousarray(W1_e.T)                                # [E, E]
    def onehot_shift(vec):
        g = np.zeros((E, 2 * T), f32)
        g[:, T - 1] = vec
        return g.astype(BF_NP)
    w2g = onehot_shift(attn_W2[0])
    gfc = onehot_shift(fc_W[0, :E])
    gfin = onehot_shift(fcf_W[0, D:])
    fc_wy = float(fc_W[0, E])
    wfh = 0.5 * fcf_W[0, :D][:, None]                                 # [D, 1]

    scales = np.array([0.5, 0.5, 1.0, 0.5], f32)
    gate_scale = np.repeat(scales, D)                                 # [4D]
    whh = (0.5 * W_hh.T) * gate_scale[None, :]                        # [D, 4D]
    bs = b_ih + b_hh + W_ih[:, 0] * float(fc_b[0])                    # [4D]
    wihb = np.stack([W_ih[:, 0] * gate_scale,
                     W_ih[:, 0] * gate_scale,
                     bs * gate_scale], axis=0)                        # [3, 4D]
    b1 = np.asarray(inputs["attn_b1"], f32)[:, None]

    weights = {
        "w1hc": w1hc.astype(BF_NP), "wke": wke.astype(BF_NP),
        "w2g": w2g, "gfc": gfc, "gfin": gfin, "whh": whh.astype(BF_NP),
        "wihb": wihb.astype(BF_NP),
        "b1": b1.astype(f32),
        "wfh": wfh.astype(BF_NP),
        "ident": np.eye(128, dtype=f32).astype(BF_NP),
    }

    x_full = np.ascontiguousarray(np.asarray(inputs["input_encoded"], f32))
    yh_full = np.asarray(inputs["y_history"], f32)[:, :, 0]           # [B_FULL, 127]

    in_maps = []
    for i in range(NCORES):
        sl = slice(i * B, (i + 1) * B)
        m = dict(weights)
        m["x"] = x_full[sl]
        m["yfc"] = np.ascontiguousarray(
            (fc_wy * yh_full[sl]).T).astype(BF_NP)                    # [127, B]
        in_maps.append(m)
    return in_maps, float(fcf_b[0])


_RUN_KW = {}


def _kernel_impl(inputs, n_steps):
    in_maps, fcf_b = _prep_host(inputs, n_steps)
    nc = _build(fcf_b, n_steps)
    res = run_bass_kernel_spmd(nc, in_maps, core_ids=list(range(NCORES)),
                               **_RUN_KW)
    out = np.concatenate(
        [np.asarray(res.results[i]["out"], np.float32).reshape(B, 1)
         for i in range(NCORES)], axis=0)
    return out, res


def kernel(**inputs) -> np.ndarray:
    out, _ = _kernel_impl(inputs, TSTEPS)
    return out
